# revision 29
# baseline (speedup 1.0000x reference)
"""Multi-head attention (B=2, N=4096, C=512, H=8, D=64) on 8 TRN2 NeuronCores.

Sharding: data-parallel over batch (2 groups of 4 cores) x tensor-parallel over
heads (2 heads/core). Per core: qkv projection, attention for its 2 heads, and
a partial output projection; the host sums the 4 per-batch partials,
transposes, adds bias.  HW exec: 409us baseline -> ~287us this version.

Design (engine-level learnings from perfetto/ntff traces):
- S = K^T Q row-packed: the two heads' S matmuls contract disjoint 64-row
  strips of the merged kt/qt tiles (tile_position (0,0)/(64,0) via base
  partitions), so they run CONCURRENTLY on the PE array - S cost halves.
- Hybrid exp: ScalarE ACT exp is a hard floor (33.5M exps = 218us/core), so
  head0 scores go exact-exp on ScalarE from [128,1024] PSUM tiles while
  head1's use a one-instruction VectorE Schraudolph:
  es_fp16 = bitcast_fp16(int16(s*A16 + B16)), max elem rel err ~3%, which the
  softmax ratio mostly cancels (end-to-end 7.0e-3).  ALL head1 chunks stay on
  VectorE: a uniform psB consumer recycles the two S-psum slots at a
  predictable pace (routing every 8th chunk to ScalarE returned those slots
  late behind the exp-A ACT queue and cost ~14us).
- PV keeps the ones-column denominator (lhsT = [V_h | 1], M=65, fp16 - fp16
  streams at bf16 rate).  Two concurrent matmuls into ONE psum bank corrupt
  on HW (start_tensor_calc clears the whole bank's has_written bits), so PV
  stays one matmul per (chunk, head).
- Software-pipelined global stream: PV trails S/exp by LAG=8 A-tiles with
  deep es rings; norm(nb) and proj(nb-1) are emitted at the PV-tail boundary
  inside nb+1's stream, so every PE FIFO entry is promptly executable
  (proj inputs are a full block old) and the po-ring turnover is off the
  critical path.
- Norm chain: ScalarE yu copy -> sync-queue row DMA (partition 64 -> 0) ->
  GpSimd partition_broadcast -> VectorE reciprocal/muls.  GpSimd runs ONLY
  partition_broadcast: mixing Q7 op families forces a ~6us ucode reload
  (MODIFY_POOL_CONFIG) per switch, which serialized every block boundary.
- DMAs: x^T loaded in 512-col blocks in first-use order; norm/a1 DMAs are
  enqueued ahead of the proj out-stores on the sync queue (HWDGE queues are
  FIFO; a waiting head DMA convoys everything behind it).  Scalar-queue DMAs
  are avoided: the trigger blocks the ScalarE FIFO until the transfer ends.
- Prologue halves are split with nb0's first 8 S/exp tiles emitted between
  them (PV deferred via the es rings) to fill the x-DMA shadow.
- PSUM (16KB/partition, exact): psA h0 [128,1024]x2 + psB h1 [128,512]x2 +
  shared [128,512]x2 ring (prologue qkv / PV accumulators po0,po1 / proj pp).
"""
import os
import sys

for _p in ("/opt/trn_rl_repo", "/root/.axon_site/_ro/trn_rl_repo"):
    if os.path.isdir(_p) and _p not in sys.path:
        sys.path.append(_p)

import numpy as np
import ml_dtypes
from contextlib import ExitStack

import concourse.bass as bass
import concourse.mybir as mybir
import concourse.tile as tile
from concourse import bacc
from concourse.bass_utils import run_bass_kernel_spmd

F32 = mybir.dt.float32
BF16 = mybir.dt.bfloat16
FP16 = mybir.dt.float16
I16 = mybir.dt.int16
EXP = mybir.ActivationFunctionType.Exp
COPY = mybir.ActivationFunctionType.Copy
MULT = mybir.AluOpType.mult
ADD = mybir.AluOpType.add

DIM, N, HD = 512, 4096, 64
SCALE = HD ** -0.5
NB = N // 512     # 8  n-blocks of 512 queries
MB = N // 128     # 32 m-chunks of 128 keys
CC = DIM // 128   # 4  c-chunks of the model dim
NT = MB // 2      # 16 A-tiles of 2 chunks per n-block
LAG = 8           # PV trails S/exp by this many A-tiles

# Schraudolph fp16 exp: bitcast_fp16(int16(s*A16 + B16)) ~= exp(s*SCALE)
A16 = float(SCALE * np.log2(np.e) * 1024.0)
B16 = float(15 * 1024 - 44.24)

NAME_MAP = {}  # instruction name -> semantic label (for trace analysis)


def lab(inst, label):
    try:
        NAME_MAP[inst.ins.name] = label
    except Exception:
        try:
            NAME_MAP[inst.name] = label
        except Exception:
            pass
    return inst


def build_nc():
    nc = bacc.Bacc("TRN2", target_bir_lowering=False)
    xT = nc.declare_dram_parameter("xT", [DIM, N], BF16, isOutput=False)
    wqkvT = nc.declare_dram_parameter("wqkvT", [DIM, 384], BF16, isOutput=False)
    wpT = nc.declare_dram_parameter("wpT", [128, DIM], BF16, isOutput=False)
    out = nc.declare_dram_parameter("out", [DIM, N], F32, isOutput=True)

    with ExitStack() as ctx:
        tc = ctx.enter_context(tile.TileContext(nc))
        big = ctx.enter_context(tc.tile_pool(name="big", bufs=1))
        eap = ctx.enter_context(tc.tile_pool(name="eap", bufs=14))
        ebp = ctx.enter_context(tc.tile_pool(name="ebp", bufs=28))
        yup = ctx.enter_context(tc.tile_pool(name="yup", bufs=2))
        ysp = ctx.enter_context(tc.tile_pool(name="ysp", bufs=8))
        psA = ctx.enter_context(tc.tile_pool(name="psA", bufs=2, space="PSUM"))
        psB = ctx.enter_context(tc.tile_pool(name="psB", bufs=2, space="PSUM"))
        pop = ctx.enter_context(tc.tile_pool(name="pop", bufs=2, space="PSUM"))

        # ---- weight loads + x^T in 512-col blocks, in use order ----
        wq = []
        for cc in range(CC):
            tb = big.tile([128, 384], BF16, tag=f"wqb{cc}", name=f"wqb{cc}")
            nc.sync.dma_start(out=tb[:], in_=wqkvT[cc * 128:(cc + 1) * 128, :])
            wq.append(tb)
        wpb = big.tile([128, DIM], BF16, tag="wpb", name="wpb")
        nc.sync.dma_start(out=wpb[:], in_=wpT[:, :])
        xtb = []
        for cc in range(CC):
            tb = big.tile([128, N], BF16, tag=f"xtb{cc}", name=f"xtb{cc}")
            xtb.append(tb)
        for nbq in range(NB):
            ns = slice(nbq * 512, (nbq + 1) * 512)
            for cc in range(CC):
                nc.sync.dma_start(
                    out=xtb[cc][:, ns], in_=xT[cc * 128:(cc + 1) * 128, ns]
                )

        # V2 layout per m-chunk: [V_h0(64) | 1 | V_h1(64) | 1], fp16
        v2 = big.tile([128, 130 * MB], FP16, tag="v2", name="v2")
        nc.vector.memset(v2[:], 1.0)

        qt = big.tile([128, N], BF16, tag="qt", name="qt")
        # merged K^T: partitions 0:64 = head0 dims, 64:128 = head1 dims
        kt = big.tile([128, N], BF16, tag="kt", name="kt")
        atB = big.tile([128, N], BF16, tag="atB", name="atB")

        def emit_kq(blk, nb):
            # blk 1 = K rows of wqkv, blk 0 = Q rows
            ps = pop.tile([128, 512], F32, tag="po", name="po")
            for cc in range(CC):
                nc.tensor.matmul(
                    ps[:],
                    lhsT=wq[cc][:, blk * 128:(blk + 1) * 128],
                    rhs=xtb[cc][:, nb * 512:(nb + 1) * 512],
                    start=(cc == 0),
                    stop=(cc == CC - 1),
                )
            ns = slice(nb * 512, (nb + 1) * 512)
            dst = qt if blk == 0 else kt
            lab(nc.scalar.activation(out=dst[:, ns], in_=ps[:], func=COPY),
                f"kqcp{blk}n{nb}")

        def emit_v(mb):
            ps = pop.tile([128, 512], F32, tag="po", name="po")
            for cc in range(CC):
                nc.tensor.matmul(
                    ps[:, 0:128],
                    lhsT=xtb[cc][:, mb * 128:(mb + 1) * 128],
                    rhs=wq[cc][:, 256:384],
                    start=(cc == 0),
                    stop=(cc == CC - 1),
                )
            # one strided copy: [h0 64 | gap | h1 64] into the 130-wide block
            src = ps[:, 0:128].rearrange("p (b c) -> p b c", c=64)
            dst = v2[:, mb * 130:mb * 130 + 130].rearrange("p (b c) -> p b c", c=65)[
                :, :, 0:64
            ]
            nc.scalar.activation(out=dst, in_=src, func=COPY)

        def emit_proj(nb):
            ns = slice(nb * 512, (nb + 1) * 512)
            for ob in range(4):
                pp = pop.tile([128, 512], F32, tag="po", name="pp")
                lab(nc.tensor.matmul(
                    pp[:],
                    lhsT=wpb[:, ob * 128:(ob + 1) * 128],
                    rhs=atB[:, ns],
                    start=True,
                    stop=True,
                ), f"prj{nb}o{ob}")
                ys = ysp.tile([128, 512], F32, tag="ys", name="ys")
                lab(nc.scalar.activation(out=ys[:], in_=pp[:], func=COPY),
                    f"ysc{nb}o{ob}")
                lab(nc.sync.dma_start(out=out[ob * 128:(ob + 1) * 128, ns],
                                      in_=ys[:]), f"odma{nb}o{ob}")

        def emit_norm(nb, h, po):
            ns = slice(nb * 512, (nb + 1) * 512)
            yu = yup.tile([128, 512], F32, tag="yu", name="yu")
            den = yup.tile([128, 512], F32, tag="den", name="den")
            rec = yup.tile([128, 512], F32, tag="rec", name="rec")
            if h == 0:
                # po[0:65] = [y_h0 | d_h0]; d must hop to partition 0 by DMA
                lab(nc.scalar.activation(out=yu[0:65, :], in_=po[0:65, :], func=COPY),
                    f"yu{nb}h0")
                row = yup.tile([1, 512], F32, tag="row", name="row")
                lab(nc.sync.dma_start(out=row[:], in_=yu[64:65, :]), f"rowdma{nb}")
                lab(nc.gpsimd.partition_broadcast(den[0:64, :], row[0:1, :]),
                    f"bcast{nb}h0")
                lab(nc.vector.reciprocal_approx_fast(out=rec[0:64, :], in_=den[0:64, :]),
                    f"rec{nb}h0")
                lab(nc.vector.tensor_mul(out=atB[0:64, ns], in0=yu[0:64, :],
                                         in1=rec[0:64, :]), f"mul{nb}h0")
            else:
                lab(nc.scalar.activation(out=yu[0:65, :], in_=po[0:65, :], func=COPY),
                    f"yu{nb}h1")
                row = yup.tile([1, 512], F32, tag="row", name="row")
                lab(nc.sync.dma_start(out=row[:], in_=yu[64:65, :]), f"rowdma{nb}h1")
                lab(nc.gpsimd.partition_broadcast(den[0:64, :], row[0:1, :]),
                    f"bcast{nb}h1")
                lab(nc.vector.reciprocal_approx_fast(out=rec[0:64, :],
                                                     in_=den[0:64, :]),
                    f"rec{nb}h1")
                a1 = yup.tile([64, 512], BF16, tag="a1", name="a1")
                lab(nc.vector.tensor_mul(out=a1[:], in0=yu[0:64, :],
                                         in1=rec[0:64, :]), f"mul{nb}h1")
                lab(nc.sync.dma_start(out=atB[64:128, ns], in_=a1[:]), f"a1dma{nb}")

        # ---- attention machinery ----
        def emit_s_exp(nb, t, ea, eb):
            ns = slice(nb * 512, (nb + 1) * 512)
            pa = psA.tile([128, 1024], F32, tag="pa", name="pa")
            pb = [psB.tile([128, 512], F32, tag="pb", name="pb") for _ in range(2)]
            for j in range(2):
                mb = 2 * t + j
                ms = slice(mb * 128, (mb + 1) * 128)
                lab(nc.tensor.matmul(
                    pa[:, j * 512:(j + 1) * 512],
                    lhsT=kt[0:64, ms],
                    rhs=qt[0:64, ns],
                    start=True,
                    stop=True,
                ), f"S{nb}t{t}j{j}h0")
                lab(nc.tensor.matmul(
                    pb[j][:],
                    lhsT=kt[64:128, ms],
                    rhs=qt[64:128, ns],
                    start=True,
                    stop=True,
                ), f"S{nb}t{t}j{j}h1")
            e = eap.tile([128, 1024], FP16, tag="ea", name="ea")
            lab(nc.scalar.activation(out=e[:], in_=pa[:], func=EXP, scale=SCALE),
                f"expA{nb}t{t}")
            ea[t] = e
            for j in range(2):
                mb = 2 * t + j
                b = ebp.tile([128, 512], FP16, tag="eb", name="eb")
                # uniform consumer: every h1 chunk on VectorE, so psB slots
                # recycle at a predictable DVE pace (mixed ScalarE consumers
                # returned slots late behind the exp-A ACT queue)
                lab(nc.vector.tensor_scalar(
                    out=b[:].bitcast(I16),
                    in0=pb[j][:],
                    scalar1=A16,
                    scalar2=B16,
                    op0=MULT,
                    op1=ADD,
                ), f"ts{nb}m{mb}")
                eb[mb] = b

        def emit_pv(t, ea, eb, po0, po1):
            for j in range(2):
                mb = 2 * t + j
                lab(nc.tensor.matmul(
                    po0[0:65, :],
                    lhsT=v2[:, mb * 130:mb * 130 + 65],
                    rhs=ea[t][:, j * 512:(j + 1) * 512],
                    start=(mb == 0),
                    stop=(mb == MB - 1),
                ), f"PV{t}m{mb}h0")
                lab(nc.tensor.matmul(
                    po1[0:65, :],
                    lhsT=v2[:, mb * 130 + 65:mb * 130 + 130],
                    rhs=eb[mb][:],
                    start=(mb == 0),
                    stop=(mb == MB - 1),
                ), f"PV{t}m{mb}h1")

        # ---- prologue half 0 with early nb0 attention interleaved ----
        # S(0,t) only needs K tokens 256t..256t+255 (kq(1, t//2)) and Q block
        # 0, so the first exp fires ~15us earlier than a serial prologue.
        es = {0: ([None] * NT, [None] * MB)}
        emit_kq(1, 0)
        emit_kq(0, 0)
        for kb in range(4):
            if kb >= 1:
                emit_kq(1, kb)
            emit_s_exp(0, 2 * kb, *es[0])
            emit_s_exp(0, 2 * kb + 1, *es[0])
        for mb in range(16):
            emit_v(mb)
        for nb in range(1, 4):
            emit_kq(0, nb)

        for nb in range(4, 8):
            emit_kq(1, nb)
        for mb in range(16, 32):
            emit_v(mb)
        for nb in range(4, 8):
            emit_kq(0, nb)

        # ---- software-pipelined global stream ----
        # Stream position g runs S+exp for (nb, t) = divmod(g, NT).  PV for
        # (nb, t<=12) is emitted at position 16nb+LAG+3+t; PV t=13,14 both at
        # +16 and t=15 at +17, immediately followed by boundary work:
        # norm(nb), proj(nb-1) (inputs one block old -> never stalls the PE
        # FIFO), and the po ring turnover for nb+1.  Everything an engine
        # dequeues is executable promptly.
        po = {
            0: (
                pop.tile([128, 512], F32, tag="po", name="po0"),
                pop.tile([128, 512], F32, tag="po", name="po1"),
            )
        }

        def pv_sched(g):
            # list of (nb, t) PV emissions due at stream position g
            out_ = []
            for nb in range(max(0, (g - LAG - 18) // NT), NB):
                s0 = NT * nb + (6 if nb == NB - 1 else LAG)
                for t in range(13):
                    if s0 + 3 + t == g:
                        out_.append((nb, t))
                if s0 + 16 == g:
                    out_ += [(nb, 13), (nb, 14)]
                if s0 + 17 == g:
                    out_.append((nb, 15))
            return out_

        LAST_G = NT * (NB - 1) + 6 + 17
        for g in range(8, LAST_G + 1):
            nb, t = divmod(g, NT)
            if nb < NB:
                if t == 0:
                    es[nb] = ([None] * NT, [None] * MB)
                emit_s_exp(nb, t, *es[nb])
            for pnb, pt in pv_sched(g):
                emit_pv(pt, *es[pnb], *po[pnb])
                if pt == 15:
                    emit_norm(pnb, 0, po[pnb][0])
                    emit_norm(pnb, 1, po[pnb][1])
                    if pnb >= 1:
                        emit_proj(pnb - 1)
                    del es[pnb]
                    if pnb + 1 < NB:
                        po[pnb + 1] = (
                            pop.tile([128, 512], F32, tag="po", name="po0"),
                            pop.tile([128, 512], F32, tag="po", name="po1"),
                        )
        emit_proj(NB - 1)

    nc.compile()
    return nc


_NC_CACHE = None
LAST_EXEC_NS = None


def kernel(x, w_qkv, w_proj, b_proj):
    global _NC_CACHE, LAST_EXEC_NS
    x = np.ascontiguousarray(np.asarray(x, dtype=np.float32))
    w_qkv = np.asarray(w_qkv, dtype=np.float32)
    w_proj = np.asarray(w_proj, dtype=np.float32)
    b_proj = np.asarray(b_proj, dtype=np.float32)
    B = x.shape[0]

    if _NC_CACHE is None:
        _NC_CACHE = build_nc()
    nc = _NC_CACHE

    bf16 = ml_dtypes.bfloat16
    xTs = [np.ascontiguousarray(x[b].T.astype(bf16)) for b in range(B)]
    in_maps = []
    for c in range(8):
        b, hp = c // 4, c % 4
        qr = w_qkv[2 * hp * 64:2 * hp * 64 + 128]
        kr = w_qkv[512 + 2 * hp * 64:512 + 2 * hp * 64 + 128]
        vr = w_qkv[1024 + 2 * hp * 64:1024 + 2 * hp * 64 + 128]
        wqkvT = np.ascontiguousarray(
            np.concatenate([qr, kr, vr], 0).T.astype(bf16)
        )
        wpT = np.ascontiguousarray(w_proj[:, hp * 128:(hp + 1) * 128].T.astype(bf16))
        in_maps.append({"xT": xTs[b], "wqkvT": wqkvT, "wpT": wpT})

    res = run_bass_kernel_spmd(
        nc,
        in_maps,
        core_ids=list(range(8)),
        trace=bool(int(os.environ.get("ATTN_TRACE", "0"))),
    )
    LAST_EXEC_NS = res.exec_time_ns

    out = np.zeros((B, N, DIM), np.float32)
    for b in range(B):
        acc = res.results[4 * b]["out"].copy()
        for c in range(4 * b + 1, 4 * b + 4):
            acc += res.results[c]["out"]
        out[b] = acc.T + b_proj
    return out
